# revision 1
# baseline (speedup 1.0000x reference)
"""Modulated Conv2D (StyleGAN2-style) Trainium2 Bass kernel.

Problem shapes (hardcoded):
  x: [16, 256, 64, 64] f32    y: [16, 512] f32
  weights: [256, 256, 3, 3]   bias: [256]
  style_w: [256, 512]         style_b: [256]
  out: [16, 256, 64, 64] f32

Math identity used: instead of materializing per-sample modulated weights,
  out[b,o] = (1/wstd[b,o]) * conv(x[b] * style[b,:], w)[o] + bias[o]
  wstd[b,o] = sqrt(sum_i W2[o,i] * style[b,i]^2 + eps),  W2[o,i] = sum_kk w[o,i,kk]^2
so the conv weights are batch-independent (shared across samples/cores).

Sharding: data-parallel over batch, 2 samples per core across 8 cores.
Conv computed as 9 shifted matmuls (per 3x3 tap) accumulating in PSUM,
bf16 operands with f32 accumulation.

Engine assignment: PE transposes weights (f32) + conv matmuls; ScalarE does
the transpose-PSUM drains (f32->bf16) and the output scale+bias; DVE does
style and the x scale-casts; GPSIMD does W2 = sum w^2 and pad memsets.
"""

import numpy as np

import concourse.bass as bass
import concourse.tile as tile
from concourse import bacc, mybir
from concourse import bass_utils
from concourse.masks import make_identity

EPS = 1e-8
P = 128
B_LOC = 2          # samples per core
CIN, COUT = 256, 256
NI, NO = CIN // P, COUT // P   # 2, 2
S = 512
H = W = 64
KK = 9             # 3x3 taps
HP, WP = H + 2, W + 2  # zero-padded image
N_CORES = 8
ROWS_A = 34        # first-chunk rows of the x load/cast split (covers half0 reads)

F32 = mybir.dt.float32
BF16 = mybir.dt.bfloat16
AF = mybir.ActivationFunctionType

# HW-bisection flags (all True = fastest per TimelineSim; flipped off to
# isolate hardware-only failures)
import os as _os
GROUPED_T = _os.environ.get("K_GROUPED_T", "0") == "1"   # multi-transpose per PSUM bank
MID_T01 = _os.environ.get("K_MID_T01", "0") == "1"       # T01 inside first mm block
USE_TTR = _os.environ.get("K_USE_TTR", "0") == "1"       # TensorTensorReduce for style
VEC_OUT = _os.environ.get("K_VEC_OUT", "0") == "1"       # DVE drain for last blocks
FINE_TAIL = _os.environ.get("K_FINE_TAIL", "0") == "1"   # eighth-sized tail blocks


def build_conv2dmod(nc):
    x = nc.dram_tensor("x", [B_LOC, CIN, H, W], F32, kind="ExternalInput")
    y = nc.dram_tensor("y", [B_LOC, S], F32, kind="ExternalInput")
    weights = nc.dram_tensor("weights", [COUT, CIN, 3, 3], F32, kind="ExternalInput")
    bias = nc.dram_tensor("bias", [COUT], F32, kind="ExternalInput")
    style_w = nc.dram_tensor("style_w", [CIN, S], F32, kind="ExternalInput")
    style_b = nc.dram_tensor("style_b", [CIN], F32, kind="ExternalInput")
    out = nc.dram_tensor("out", [B_LOC, COUT, H, W], F32, kind="ExternalOutput")

    with tile.TileContext(nc) as tc:
        with (
            tc.tile_pool(name="consts", bufs=1) as consts,
            tc.tile_pool(name="temps", bufs=2) as temps,
            tc.tile_pool(name="xin_pool", bufs=1) as xin_pool,
            tc.tile_pool(name="xs_pool", bufs=1) as xs_pool,
            tc.tile_pool(name="out_pool", bufs=3) as out_pool,
            tc.tile_pool(name="psum", bufs=2, space="PSUM") as psum,
        ):
            # ---------- DMA loads, split + ordered by when they gate compute -----
            sw_nat = [consts.tile([P, S], F32, name=f"sw_nat{i}", tag=f"sw_nat{i}")
                      for i in range(NI)]
            y_bcast = consts.tile([P, B_LOC, S], F32)
            w_nat = [consts.tile([P, CIN, KK], F32, name=f"w_nat{o}", tag=f"w_nat{o}")
                     for o in range(NO)]
            xin = {}
            for b in range(B_LOC):
                for it in range(NI):
                    xin[(b, it)] = xin_pool.tile([P, H, W], F32,
                                                 name=f"xin{b}_{it}", tag=f"xin{b}_{it}")

            def load_xin(b, it, part):
                r = slice(0, ROWS_A) if part == 0 else slice(ROWS_A, H)
                nc.sync.dma_start(xin[(b, it)][:, r, :],
                                  x.ap()[b, it * P:(it + 1) * P, r, :])

            def load_w(ot, ih):
                nc.sync.dma_start(
                    w_nat[ot][:, ih * P:(ih + 1) * P, :],
                    weights.ap()[ot * P:(ot + 1) * P, ih * P:(ih + 1) * P]
                    .rearrange("o i kh kw -> o i (kh kw)"),
                )

            load_w(0, 0)
            nc.sync.dma_start(y_bcast[:], y.ap()[None].to_broadcast((P, B_LOC, S)))
            nc.sync.dma_start(sw_nat[0][:], style_w.ap()[0:P, :])
            bias_col = consts.tile([P, NO], F32)
            nc.sync.dma_start(bias_col[:], bias.ap().rearrange("(oo oi) -> oi oo", oi=P))
            style_b_col = consts.tile([P, NI], F32)
            nc.sync.dma_start(style_b_col[:], style_b.ap().rearrange("(io ii) -> ii io", ii=P))
            load_xin(0, 0, 0)
            nc.sync.dma_start(sw_nat[1][:], style_w.ap()[P:2 * P, :])
            load_w(0, 1)
            load_xin(0, 1, 0)
            load_xin(0, 0, 1)
            load_xin(0, 1, 1)
            load_w(1, 0)
            load_w(1, 1)
            for it in range(NI):
                for part in range(2):
                    load_xin(1, it, part)

            # ---------- GPSIMD: identity, y broadcast, b0 pad borders ----------
            identity_bf = consts.tile([P, P], BF16)
            make_identity(nc, identity_bf)

            # pre-warm the ACT function table that Sqrt lives in, so the
            # LoadActFuncSet doesn't fire mid-kernel on the demod path
            eps_col = consts.tile([P, 1], F32)
            nc.gpsimd.memset(eps_col[:], EPS)
            lafs_warm = consts.tile([P, 1], F32)
            nc.scalar.activation(lafs_warm[:], eps_col[:], AF.Sqrt)

            xs = {}

            def xs_borders(b):
                for it in range(NI):
                    xp = xs_pool.tile([P, HP, WP], BF16, name=f"xs{b}_{it}",
                                      tag=f"xs{b}_{it}")
                    nc.gpsimd.memset(xp[:, 0, :], 0.0)
                    nc.gpsimd.memset(xp[:, HP - 1, :], 0.0)
                    nc.gpsimd.memset(xp[:, 1:HP - 1, 0], 0.0)
                    nc.gpsimd.memset(xp[:, 1:HP - 1, WP - 1], 0.0)
                    xs[(b, it)] = xp

            xs_borders(0)

            # ---------- style (DVE): fused multiply-reduce per (it, b) ----------
            style_col = []
            style2 = []
            for it in range(NI):
                sc = consts.tile([P, B_LOC], F32, name=f"style_col{it}", tag=f"style_col{it}")
                s2 = consts.tile([P, B_LOC], F32, name=f"style2{it}", tag=f"style2{it}")
                if USE_TTR:
                    for b in range(B_LOC):
                        junk = temps.tile([P, S], F32, name=f"junk{it}_{b}", tag="junk")
                        nc.vector.tensor_tensor_reduce(
                            out=junk[:], in0=sw_nat[it][:], in1=y_bcast[:, b, :],
                            scale=1.0, scalar=0.0,
                            op0=mybir.AluOpType.mult, op1=mybir.AluOpType.add,
                            accum_out=sc[:, b:b + 1],
                        )
                        nc.vector.tensor_scalar_add(sc[:, b:b + 1], sc[:, b:b + 1],
                                                    style_b_col[:, it:it + 1])
                        nc.vector.tensor_mul(s2[:, b:b + 1], sc[:, b:b + 1],
                                             sc[:, b:b + 1])
                else:
                    tmp = temps.tile([P, B_LOC, S], F32, name=f"stmp{it}", tag="junk")
                    nc.vector.tensor_mul(
                        tmp[:], y_bcast[:],
                        sw_nat[it][:, None, :].to_broadcast((P, B_LOC, S)))
                    nc.vector.reduce_sum(sc[:], tmp[:], axis=mybir.AxisListType.X)
                    nc.vector.tensor_scalar_add(sc[:], sc[:], style_b_col[:, it:it + 1])
                    nc.vector.tensor_mul(s2[:], sc[:], sc[:])
                style_col.append(sc)
                style2.append(s2)

            # ---------- weights: bf16 cast (GPSIMD) + PE transpose + ACT drain ----
            w_nat_bf = [consts.tile([P, CIN, KK], BF16, name=f"w_nbf{o}", tag=f"w_nbf{o}")
                        for o in range(NO)]
            # kk-major so the PSUM drain and the matmul lhsT reads are contiguous
            w_bf = [consts.tile([P, KK, COUT], BF16, name=f"w_bf{i}", tag=f"w_bf{i}")
                    for i in range(NI)]
            tp_idx = [0]

            def cast_w(ot, ih, eng=None):
                (eng or nc.gpsimd).tensor_copy(
                    w_nat_bf[ot][:, ih * P:(ih + 1) * P, :],
                    w_nat[ot][:, ih * P:(ih + 1) * P, :])

            def transpose_wblock(ot, it):
                # w_nat_bf[ot][:, it_block, kk] --PE--> psum[i, o] --ACT--> w_bf
                if GROUPED_T:
                    # 4 transposes share one PSUM bank, drain in a single ACT copy
                    groups = ((0, 4), (4, 4), (8, 1))
                else:
                    groups = tuple((kk, 1) for kk in range(KK))
                for kk0, n in groups:
                    pt = psum.tile([P, n, P], BF16, name=f"tp{ot}_{it}_{kk0}",
                                   tag=f"ch{tp_idx[0] % 4}")
                    tp_idx[0] += 1
                    for j in range(n):
                        nc.tensor.transpose(
                            pt[:, j, :],
                            w_nat_bf[ot][:, it * P:(it + 1) * P, kk0 + j],
                            identity_bf[:],
                        )
                    nc.scalar.copy(
                        w_bf[it][:, kk0:kk0 + n, ot * P:(ot + 1) * P], pt[:]
                    )

            cast_w(0, 0, nc.vector)
            transpose_wblock(0, 0)

            # ---------- x scale+cast (DVE), ordered by need ----------
            def xs_cast(b, it, part):
                r = slice(0, ROWS_A) if part == 0 else slice(ROWS_A, H)
                return nc.vector.tensor_scalar_mul(
                    xs[(b, it)][:, r.start + 1:r.stop + 1, 1:W + 1],
                    xin[(b, it)][:, r, :],
                    style_col[it][:, b:b + 1],
                )

            xs_cast(0, 0, 0)
            xs_cast(0, 1, 0)
            xs_cast(0, 0, 1)
            last_cast_b0 = xs_cast(0, 1, 1)

            # ---------- main conv block: 18*nchunks matmuls per call ----------
            def mm_block(b, ot, r0, nchunks, ctag0=0, mid_cb=None):
                pcs = [psum.tile([P, 8, W], F32, name=f"pc{b}{ot}{r0}_{c}",
                                 tag=f"ch{(ctag0 + c) % 4}")
                       for c in range(nchunks)]
                first, last = (0, 0), (NI - 1, KK - 1)
                for it in range(NI):
                    if it == 1 and mid_cb is not None:
                        mid_cb()
                    for kk in range(KK):
                        dy, dx = kk // 3, kk % 3
                        lhsT = w_bf[it][:, kk, ot * P:(ot + 1) * P]
                        for c in range(nchunks):
                            rs = r0 + c * 8 + dy
                            nc.tensor.matmul(
                                pcs[c][:], lhsT, xs[(b, it)][:, rs:rs + 8, dx:dx + W],
                                start=((it, kk) == first), stop=((it, kk) == last),
                            )
                return pcs

            def out_block(b, ot, r0, pcs, engine="scalar"):
                n = len(pcs)
                oh = out_pool.tile([P, 8 * n, W], F32, name=f"oh{b}{ot}{r0}", tag="oh")
                for c in range(n):
                    if engine == "scalar":
                        nc.scalar.activation(
                            oh[:, c * 8:(c + 1) * 8, :], pcs[c][:], AF.Identity,
                            bias=bias_col[:, ot:ot + 1], scale=winv[ot][:, b:b + 1],
                        )
                    else:
                        nc.vector.tensor_scalar(
                            oh[:, c * 8:(c + 1) * 8, :], pcs[c][:],
                            winv[ot][:, b:b + 1], bias_col[:, ot:ot + 1],
                            mybir.AluOpType.mult, mybir.AluOpType.add,
                        )
                nc.sync.dma_start(
                    out.ap()[b, ot * P:(ot + 1) * P, r0:r0 + 8 * n, :], oh[:])

            def _mid_t01():
                cast_w(0, 1, nc.vector)
                transpose_wblock(0, 1)

            if MID_T01:
                pcs_h0 = mm_block(0, 0, 0, 4, mid_cb=_mid_t01)
            else:
                _mid_t01()
                pcs_h0 = mm_block(0, 0, 0, 4)

            # ---------- demod path, emitted so the in-order PE never stalls -------
            # w^2 + kk-reduce (DVE), W2T via 4 PE transposes, sigma matmul, rsqrt
            w2_nat = []
            for ot in range(NO):
                sq = temps.tile([P, CIN, KK], F32, name=f"sq{ot}", tag="sq", bufs=1)
                sq_i = nc.vector.tensor_mul(sq[:], w_nat[ot][:], w_nat[ot][:])
                # keep the w^2 work behind the critical sample-0 casts
                bass._add_dep_helper(sq_i.ins, last_cast_b0.ins, sync=False,
                                     reason="w2 after b0 x casts")
                t = consts.tile([P, CIN], F32, name=f"w2n{ot}", tag=f"w2n{ot}")
                nc.vector.reduce_sum(t[:], sq[:], axis=mybir.AxisListType.X)
                w2_nat.append(t)

            cast_w(1, 0, nc.vector)
            transpose_wblock(1, 0)
            cast_w(1, 1, nc.vector)
            transpose_wblock(1, 1)

            identity_f = consts.tile([P, P], F32)
            make_identity(nc, identity_f)
            w2t = [consts.tile([P, COUT], F32, name=f"w2t{i}", tag=f"w2t{i}")
                   for i in range(NI)]
            for it in range(NI):
                pt = psum.tile([P, NO, P], F32, name=f"w2tp{it}",
                               tag=f"ch{tp_idx[0] % 4}")
                tp_idx[0] += 1
                for ot in range(NO):
                    nc.tensor.transpose(pt[:, ot, :],
                                        w2_nat[ot][:, it * P:(it + 1) * P],
                                        identity_f[:])
                nc.scalar.copy(w2t[it][:], pt[:].rearrange("p o i -> p (o i)"))

            winv = []
            for ot in range(NO):
                ps = psum.tile([P, B_LOC], F32, name=f"sig{ot}", tag=f"ch{ot}")
                for it in range(NI):
                    nc.tensor.matmul(
                        ps[:], w2t[it][:, ot * P:(ot + 1) * P], style2[it][:],
                        start=(it == 0), stop=(it == NI - 1),
                    )
                wstd = consts.tile([P, B_LOC], F32, name=f"wstd{ot}", tag=f"wstd{ot}")
                nc.scalar.activation(wstd[:], ps[:], AF.Sqrt, bias=eps_col[:])
                wi = consts.tile([P, B_LOC], F32, name=f"winv{ot}", tag=f"winv{ot}")
                nc.vector.reciprocal(wi[:], wstd[:])
                winv.append(wi)

            # ---------- rest of the schedule ----------
            out_block(0, 0, 0, pcs_h0)
            out_block(0, 0, 32, mm_block(0, 0, 32, 4))
            for half in range(2):
                out_block(0, 1, half * 32, mm_block(0, 1, half * 32, 4))

            # sample 1 input stage
            xs_borders(1)
            for it in range(NI):
                for part in range(2):
                    xs_cast(1, it, part)

            for half in range(2):
                out_block(1, 0, half * 32, mm_block(1, 0, half * 32, 4))
            out_block(1, 1, 0, mm_block(1, 1, 0, 4))
            # final blocks shrink progressively so the drain tail is short;
            # optionally the last two drain on DVE so ACT and DVE overlap
            tail_eng = "vector" if VEC_OUT else "scalar"
            if FINE_TAIL:
                out_block(1, 1, 32, mm_block(1, 1, 32, 2, ctag0=0))
                out_block(1, 1, 48, mm_block(1, 1, 48, 1, ctag0=2), engine=tail_eng)
                out_block(1, 1, 56, mm_block(1, 1, 56, 1, ctag0=3), engine=tail_eng)
            else:
                out_block(1, 1, 32, mm_block(1, 1, 32, 4), engine=tail_eng)
    return nc


_CACHED_NC = None


def _get_nc():
    global _CACHED_NC
    if _CACHED_NC is None:
        nc = bacc.Bacc("TRN2", target_bir_lowering=False, debug=False,
                       num_devices=N_CORES)
        build_conv2dmod(nc)
        nc.compile()
        _CACHED_NC = nc
    return _CACHED_NC


def kernel(x, y, weights, bias, style_w, style_b, _trace=False):
    x = np.ascontiguousarray(np.asarray(x, dtype=np.float32))
    y = np.ascontiguousarray(np.asarray(y, dtype=np.float32))
    weights = np.ascontiguousarray(np.asarray(weights, dtype=np.float32))
    bias = np.ascontiguousarray(np.asarray(bias, dtype=np.float32))
    style_w = np.ascontiguousarray(np.asarray(style_w, dtype=np.float32))
    style_b = np.ascontiguousarray(np.asarray(style_b, dtype=np.float32))

    nc = _get_nc()
    in_maps = [
        {
            "x": np.ascontiguousarray(x[c * B_LOC:(c + 1) * B_LOC]),
            "y": np.ascontiguousarray(y[c * B_LOC:(c + 1) * B_LOC]),
            "weights": weights,
            "bias": bias,
            "style_w": style_w,
            "style_b": style_b,
        }
        for c in range(N_CORES)
    ]
    res = bass_utils.run_bass_kernel_spmd(
        nc, in_maps, core_ids=list(range(N_CORES)), trace=_trace
    )
    out = np.concatenate([r["out"] for r in res.results], axis=0)
    if _trace:
        kernel.last_results = res
    return out



# revision 2
# speedup vs baseline: 1.0804x; 1.0804x over previous
"""Modulated Conv2D (StyleGAN2-style) Trainium2 Bass kernel.

Problem shapes (hardcoded):
  x: [16, 256, 64, 64] f32    y: [16, 512] f32
  weights: [256, 256, 3, 3]   bias: [256]
  style_w: [256, 512]         style_b: [256]
  out: [16, 256, 64, 64] f32

Math identity used: instead of materializing per-sample modulated weights,
  out[b,o] = (1/wstd[b,o]) * conv(x[b] * style[b,:], w)[o] + bias[o]
  wstd[b,o] = sqrt(sum_i W2[o,i] * style[b,i]^2 + eps),  W2[o,i] = sum_kk w[o,i,kk]^2
so the conv weights are batch-independent (shared across samples/cores).

Sharding: data-parallel over batch, 2 samples per core across 8 cores.
Conv computed as 9 shifted matmuls (per 3x3 tap) accumulating in PSUM,
bf16 operands with f32 accumulation.

Layout prep (transpose of weights to [cin, kk, cout] + bf16 cast, W2^T,
style_w^T, bias/style_b column layouts) is done host-side in numpy while
sharding, so the device kernel has no weight-transpose stage: the PE runs
the style matmuls during the initial x DMA, then streams conv matmuls
back-to-back.  Engine assignment: PE style + sigma + conv matmuls; ScalarE
drains (scale+bias); DVE x scale-casts + winv; GPSIMD pad memsets.
"""

import numpy as np
import ml_dtypes

import concourse.bass as bass
import concourse.tile as tile
from concourse import bacc, mybir
from concourse import bass_utils

EPS = 1e-8
P = 128
B_LOC = 2          # samples per core
CIN, COUT = 256, 256
NI, NO = CIN // P, COUT // P   # 2, 2
S = 512
SS = S // P        # 4 style-contraction blocks
H = W = 64
KK = 9             # 3x3 taps
HP, WP = H + 2, W + 2  # zero-padded image
N_CORES = 8
ROWS_A = 34        # first-chunk rows of the x load/cast split (covers half0 reads)

F32 = mybir.dt.float32
BF16 = mybir.dt.bfloat16
AF = mybir.ActivationFunctionType


def build_conv2dmod(nc):
    x = nc.dram_tensor("x", [B_LOC, CIN, H, W], F32, kind="ExternalInput")
    yt = nc.dram_tensor("yt", [S, B_LOC], F32, kind="ExternalInput")
    wt = nc.dram_tensor("wt", [CIN, KK, COUT], BF16, kind="ExternalInput")
    w2t = nc.dram_tensor("w2t", [CIN, COUT], F32, kind="ExternalInput")
    swt = nc.dram_tensor("swt", [S, CIN], F32, kind="ExternalInput")
    bias_col_d = nc.dram_tensor("bias_col", [P, NO], F32, kind="ExternalInput")
    stb_col_d = nc.dram_tensor("stb_col", [P, NI], F32, kind="ExternalInput")
    out = nc.dram_tensor("out", [B_LOC, COUT, H, W], F32, kind="ExternalOutput")

    with tile.TileContext(nc) as tc:
        with (
            tc.tile_pool(name="consts", bufs=1) as consts,
            tc.tile_pool(name="xin_pool", bufs=1) as xin_pool,
            tc.tile_pool(name="xs_pool", bufs=1) as xs_pool,
            tc.tile_pool(name="out_pool", bufs=3) as out_pool,
            tc.tile_pool(name="psum", bufs=2, space="PSUM") as psum,
        ):
            # ---------- DMA loads, ordered by when they gate compute ----------
            y_sb = consts.tile([P, SS, B_LOC], F32)
            nc.sync.dma_start(y_sb[:], yt.ap().rearrange("(ss p) b -> p ss b", p=P))
            sw_sb = consts.tile([P, SS, CIN], F32)
            nc.sync.dma_start(sw_sb[:], swt.ap().rearrange("(ss p) i -> p ss i", p=P))
            stb_sb = consts.tile([P, NI], F32)
            nc.sync.dma_start(stb_sb[:], stb_col_d.ap())
            w_sb = [consts.tile([P, KK, COUT], BF16, name=f"w_sb{i}", tag=f"w_sb{i}")
                    for i in range(NI)]
            nc.sync.dma_start(w_sb[0][:], wt.ap()[0:P])
            bias_sb = consts.tile([P, NO], F32)
            nc.sync.dma_start(bias_sb[:], bias_col_d.ap())

            xin = {}
            for b in range(B_LOC):
                for it in range(NI):
                    xin[(b, it)] = xin_pool.tile([P, H, W], F32,
                                                 name=f"xin{b}_{it}", tag=f"xin{b}_{it}")

            def load_xin(b, it, part):
                r = slice(0, ROWS_A) if part == 0 else slice(ROWS_A, H)
                nc.sync.dma_start(xin[(b, it)][:, r, :],
                                  x.ap()[b, it * P:(it + 1) * P, r, :])

            load_xin(0, 0, 0)
            nc.sync.dma_start(w_sb[1][:], wt.ap()[P:2 * P])
            load_xin(0, 1, 0)
            w2t_sb = consts.tile([P, NI, COUT], F32)
            nc.sync.dma_start(w2t_sb[:], w2t.ap().rearrange("(it p) o -> p it o", p=P))
            load_xin(0, 0, 1)
            load_xin(0, 1, 1)
            for it in range(NI):
                for part in range(2):
                    load_xin(1, it, part)

            # ---------- GPSIMD: eps, ACT table warm, b0 pad borders ----------
            # pre-warm the ACT function table that Sqrt lives in, so the
            # LoadActFuncSet doesn't fire mid-kernel on the demod path
            eps_col = consts.tile([P, 1], F32)
            nc.gpsimd.memset(eps_col[:], EPS)
            lafs_warm = consts.tile([P, 1], F32)
            nc.scalar.activation(lafs_warm[:], eps_col[:], AF.Sqrt)

            xs = {}

            def xs_borders(b):
                for it in range(NI):
                    xp = xs_pool.tile([P, HP, WP], BF16, name=f"xs{b}_{it}",
                                      tag=f"xs{b}_{it}")
                    nc.gpsimd.memset(xp[:, 0, :], 0.0)
                    nc.gpsimd.memset(xp[:, HP - 1, :], 0.0)
                    nc.gpsimd.memset(xp[:, 1:HP - 1, 0], 0.0)
                    nc.gpsimd.memset(xp[:, 1:HP - 1, WP - 1], 0.0)
                    xs[(b, it)] = xp

            xs_borders(0)

            # ---------- style on PE: style[i,b] = sum_s sw[s,i] y[s,b] + sb ----
            style_col = []
            style2 = []
            for it in range(NI):
                ps_st = psum.tile([P, B_LOC], F32, name=f"pst{it}", tag=f"ch{it}")
                for ss in range(SS):
                    nc.tensor.matmul(
                        ps_st[:], sw_sb[:, ss, it * P:(it + 1) * P], y_sb[:, ss, :],
                        start=(ss == 0), stop=(ss == SS - 1),
                    )
                sc = consts.tile([P, B_LOC], F32, name=f"style_col{it}",
                                 tag=f"style_col{it}")
                nc.scalar.activation(sc[:], ps_st[:], AF.Identity,
                                     bias=stb_sb[:, it:it + 1])
                s2 = consts.tile([P, B_LOC], F32, name=f"style2{it}", tag=f"style2{it}")
                nc.vector.tensor_mul(s2[:], sc[:], sc[:])
                style_col.append(sc)
                style2.append(s2)

            # ---------- x scale+cast (DVE), ordered by need ----------
            def xs_cast(b, it, part):
                r = slice(0, ROWS_A) if part == 0 else slice(ROWS_A, H)
                return nc.vector.tensor_scalar_mul(
                    xs[(b, it)][:, r.start + 1:r.stop + 1, 1:W + 1],
                    xin[(b, it)][:, r, :],
                    style_col[it][:, b:b + 1],
                )

            xs_cast(0, 0, 0)
            xs_cast(0, 1, 0)
            xs_cast(0, 0, 1)
            xs_cast(0, 1, 1)

            # ---------- demod path: sigma matmul + sqrt + reciprocal ----------
            winv = []

            def emit_sigma():
                for ot in range(NO):
                    ps = psum.tile([P, B_LOC], F32, name=f"sig{ot}", tag=f"ch{2 + ot}")
                    for it in range(NI):
                        nc.tensor.matmul(
                            ps[:], w2t_sb[:, it, ot * P:(ot + 1) * P], style2[it][:],
                            start=(it == 0), stop=(it == NI - 1),
                        )
                    wstd = consts.tile([P, B_LOC], F32, name=f"wstd{ot}",
                                       tag=f"wstd{ot}")
                    nc.scalar.activation(wstd[:], ps[:], AF.Sqrt, bias=eps_col[:])
                    wi = consts.tile([P, B_LOC], F32, name=f"winv{ot}", tag=f"winv{ot}")
                    nc.vector.reciprocal(wi[:], wstd[:])
                    winv.append(wi)

            # ---------- main conv block: 18*nchunks matmuls per call ----------
            def mm_block(b, ot, r0, nchunks, ctag0=0, mid_cb=None):
                pcs = [psum.tile([P, 8, W], F32, name=f"pc{b}{ot}{r0}_{c}",
                                 tag=f"ch{(ctag0 + c) % 4}")
                       for c in range(nchunks)]
                first, last = (0, 0), (NI - 1, KK - 1)
                for it in range(NI):
                    if it == 1 and mid_cb is not None:
                        mid_cb()
                    for kk in range(KK):
                        dy, dx = kk // 3, kk % 3
                        lhsT = w_sb[it][:, kk, ot * P:(ot + 1) * P]
                        for c in range(nchunks):
                            rs = r0 + c * 8 + dy
                            nc.tensor.matmul(
                                pcs[c][:], lhsT, xs[(b, it)][:, rs:rs + 8, dx:dx + W],
                                start=((it, kk) == first), stop=((it, kk) == last),
                            )
                return pcs

            def out_block(b, ot, r0, pcs, engine="scalar"):
                n = len(pcs)
                oh = out_pool.tile([P, 8 * n, W], F32, name=f"oh{b}{ot}{r0}", tag="oh")
                for c in range(n):
                    if engine == "scalar":
                        nc.scalar.activation(
                            oh[:, c * 8:(c + 1) * 8, :], pcs[c][:], AF.Identity,
                            bias=bias_sb[:, ot:ot + 1], scale=winv[ot][:, b:b + 1],
                        )
                    else:
                        nc.vector.tensor_scalar(
                            oh[:, c * 8:(c + 1) * 8, :], pcs[c][:],
                            winv[ot][:, b:b + 1], bias_sb[:, ot:ot + 1],
                            mybir.AluOpType.mult, mybir.AluOpType.add,
                        )
                nc.sync.dma_start(
                    out.ap()[b, ot * P:(ot + 1) * P, r0:r0 + 8 * n, :], oh[:])

            # sigma emitted mid-first-block so the PE never waits on the w2t
            # DMA and winv is ready just after the first chunk drains
            pcs_h0 = mm_block(0, 0, 0, 4, mid_cb=emit_sigma)
            out_block(0, 0, 0, pcs_h0)
            out_block(0, 0, 32, mm_block(0, 0, 32, 4))
            for half in range(2):
                out_block(0, 1, half * 32, mm_block(0, 1, half * 32, 4))

            # sample 1 input stage
            xs_borders(1)
            for it in range(NI):
                for part in range(2):
                    xs_cast(1, it, part)

            for half in range(2):
                out_block(1, 0, half * 32, mm_block(1, 0, half * 32, 4))
            out_block(1, 1, 0, mm_block(1, 1, 0, 4))
            # final blocks shrink progressively so the drain tail is short;
            # the last two drain ACT/DVE in parallel
            out_block(1, 1, 32, mm_block(1, 1, 32, 2, ctag0=0))
            out_block(1, 1, 48, mm_block(1, 1, 48, 1, ctag0=2))
            out_block(1, 1, 56, mm_block(1, 1, 56, 1, ctag0=3), engine="vector")
    return nc


_CACHED_NC = None


def _get_nc():
    global _CACHED_NC
    if _CACHED_NC is None:
        nc = bacc.Bacc("TRN2", target_bir_lowering=False, debug=False,
                       num_devices=N_CORES)
        build_conv2dmod(nc)
        nc.compile()
        _CACHED_NC = nc
    return _CACHED_NC


def kernel(x, y, weights, bias, style_w, style_b, _trace=False):
    x = np.ascontiguousarray(np.asarray(x, dtype=np.float32))
    y = np.ascontiguousarray(np.asarray(y, dtype=np.float32))
    weights = np.asarray(weights, dtype=np.float32)
    bias = np.asarray(bias, dtype=np.float32)
    style_w = np.asarray(style_w, dtype=np.float32)
    style_b = np.asarray(style_b, dtype=np.float32)

    # host-side layout prep (small tensors only: ~3MB total)
    wt = np.ascontiguousarray(
        weights.transpose(1, 2, 3, 0).reshape(CIN, KK, COUT)
    ).astype(ml_dtypes.bfloat16)                                   # [i, kk, o]
    w2t = np.ascontiguousarray((weights * weights).sum(axis=(2, 3)).T)  # [i, o]
    swt = np.ascontiguousarray(style_w.T)                          # [s, i]
    bias_col = np.ascontiguousarray(bias.reshape(NO, P).T)         # [p, oo]
    stb_col = np.ascontiguousarray(style_b.reshape(NI, P).T)       # [p, io]

    nc = _get_nc()
    in_maps = [
        {
            "x": np.ascontiguousarray(x[c * B_LOC:(c + 1) * B_LOC]),
            "yt": np.ascontiguousarray(y[c * B_LOC:(c + 1) * B_LOC].T),
            "wt": wt,
            "w2t": w2t,
            "swt": swt,
            "bias_col": bias_col,
            "stb_col": stb_col,
        }
        for c in range(N_CORES)
    ]
    res = bass_utils.run_bass_kernel_spmd(
        nc, in_maps, core_ids=list(range(N_CORES)), trace=_trace
    )
    out = np.concatenate([r["out"] for r in res.results], axis=0)
    if _trace:
        kernel.last_results = res
    return out


# revision 5
# speedup vs baseline: 1.0841x; 1.0034x over previous
"""Modulated Conv2D (StyleGAN2-style) Trainium2 Bass kernel.

Problem shapes (hardcoded):
  x: [16, 256, 64, 64] f32    y: [16, 512] f32
  weights: [256, 256, 3, 3]   bias: [256]
  style_w: [256, 512]         style_b: [256]
  out: [16, 256, 64, 64] f32

Math identity used: instead of materializing per-sample modulated weights,
  out[b,o] = (1/wstd[b,o]) * conv(x[b] * style[b,:], w)[o] + bias[o]
  wstd[b,o] = sqrt(sum_i W2[o,i] * style[b,i]^2 + eps),  W2[o,i] = sum_kk w[o,i,kk]^2
so the conv weights are batch-independent (shared across samples/cores).

Sharding: data-parallel over batch, 2 samples per core across 8 cores.
Conv computed as 9 shifted matmuls (per 3x3 tap) accumulating in PSUM,
bf16 operands with f32 accumulation.

Host-side prep while sharding (all small or O(input-bytes) casts): weights
transposed to [cin, kk, cout] bf16, x cast to bf16 (halves HBM + doubles
DVE cast rate), and the per-sample scalars (style vector + demodulation
1/wstd, ~4 MMACs of the 77-GFLOP problem) folded into one packed [128,10]
f32 constant per core.  The device kernel is pure conv: DMA x over the two
HWDGE rings (inputs on the ACT ring, consts+outputs on the SYNC ring),
DVE scale-casts, PE streams 576 conv matmuls back-to-back (with a few
junk matmuls up front to lift the HAM clock gate early), ScalarE drains
scale+bias, stores overlap.
"""

import numpy as np
import ml_dtypes

import concourse.bass as bass
import concourse.tile as tile
from concourse import bacc, mybir
from concourse import bass_utils

EPS = 1e-8
P = 128
B_LOC = 2          # samples per core
CIN, COUT = 256, 256
NI, NO = CIN // P, COUT // P   # 2, 2
S = 512
H = W = 64
KK = 9             # 3x3 taps
HP, WP = H + 2, W + 2  # zero-padded image
N_CORES = 8
ROWS_A = 34        # first-chunk rows of the x load/cast split (covers half0 reads)
N_WARM_MM = 24     # junk matmuls to lift the HAM clock gate before the conv

F32 = mybir.dt.float32
BF16 = mybir.dt.bfloat16
AF = mybir.ActivationFunctionType

# packed per-core constant columns: [style(it,b) x4 | winv(ot,b) x4 | bias(ot) x2]
C_STYLE, C_WINV, C_BIAS, NCOLS = 0, 4, 8, 10


def build_conv2dmod(nc):
    xbf = nc.dram_tensor("xbf", [B_LOC, CIN, H, W], BF16, kind="ExternalInput")
    wt = nc.dram_tensor("wt", [CIN, KK, COUT], BF16, kind="ExternalInput")
    cols = nc.dram_tensor("cols", [P, NCOLS], F32, kind="ExternalInput")
    out = nc.dram_tensor("out", [B_LOC, COUT, H, W], F32, kind="ExternalOutput")

    with tile.TileContext(nc) as tc:
        with (
            tc.tile_pool(name="consts", bufs=1) as consts,
            tc.tile_pool(name="xin_pool", bufs=1) as xin_pool,
            tc.tile_pool(name="xs_pool", bufs=1) as xs_pool,
            tc.tile_pool(name="out_pool", bufs=3) as out_pool,
            tc.tile_pool(name="psum", bufs=2, space="PSUM") as psum,
        ):
            # ---------- DMA loads, split over both HWDGE rings ----------
            cols_sb = consts.tile([P, NCOLS], F32)
            nc.sync.dma_start(cols_sb[:], cols.ap())

            xin = {}
            for b in range(B_LOC):
                for it in range(NI):
                    xin[(b, it)] = xin_pool.tile([P, H, W], BF16,
                                                 name=f"xin{b}_{it}", tag=f"xin{b}_{it}")

            def load_xin(eng, b, it, part):
                r = slice(0, ROWS_A) if part == 0 else slice(ROWS_A, H)
                eng.dma_start(xin[(b, it)][:, r, :],
                              xbf.ap()[b, it * P:(it + 1) * P, r, :])

            w_sb = [consts.tile([P, KK, COUT], BF16, name=f"w_sb{i}", tag=f"w_sb{i}")
                    for i in range(NI)]
            # sync ring: first-needed x tile, weights, (later) out stores
            load_xin(nc.sync, 0, 0, 0)
            nc.sync.dma_start(w_sb[0][:], wt.ap()[0:P])
            load_xin(nc.sync, 0, 0, 1)
            nc.sync.dma_start(w_sb[1][:], wt.ap()[P:2 * P])
            # act ring: the rest of x
            load_xin(nc.scalar, 0, 1, 0)
            load_xin(nc.scalar, 0, 1, 1)
            for it in range(NI):
                for part in range(2):
                    load_xin(nc.scalar, 1, it, part)

            # ---------- GPSIMD: pad borders;  ACT: warm the func table ----------
            lafs_warm = consts.tile([P, 1], F32)
            nc.scalar.activation(lafs_warm[:], cols_sb[:, 0:1], AF.Identity,
                                 bias=cols_sb[:, 1:2], scale=cols_sb[:, 2:3])

            xs = {}

            def xs_borders(b):
                for it in range(NI):
                    xp = xs_pool.tile([P, HP, WP], BF16, name=f"xs{b}_{it}",
                                      tag=f"xs{b}_{it}")
                    nc.gpsimd.memset(xp[:, 0, :], 0.0)
                    nc.gpsimd.memset(xp[:, HP - 1, :], 0.0)
                    nc.gpsimd.memset(xp[:, 1:HP - 1, 0], 0.0)
                    nc.gpsimd.memset(xp[:, 1:HP - 1, WP - 1], 0.0)
                    xs[(b, it)] = xp

            xs_borders(0)

            # ---------- PE warm-up: junk matmuls while x streams in ----------
            warm_ps = psum.tile([1, NCOLS], F32, name="warm_ps", tag="ch0")
            for _ in range(N_WARM_MM):
                nc.tensor.matmul(warm_ps[:], cols_sb[:, 0:1], cols_sb[:],
                                 start=True, stop=True)

            # ---------- x scale+cast (DVE), ordered by need ----------
            def xs_cast(b, it, part):
                r = slice(0, ROWS_A) if part == 0 else slice(ROWS_A, H)
                return nc.vector.tensor_scalar_mul(
                    xs[(b, it)][:, r.start + 1:r.stop + 1, 1:W + 1],
                    xin[(b, it)][:, r, :],
                    cols_sb[:, C_STYLE + it * B_LOC + b:C_STYLE + it * B_LOC + b + 1],
                )

            xs_cast(0, 0, 0)
            xs_cast(0, 1, 0)
            xs_cast(0, 0, 1)
            xs_cast(0, 1, 1)

            # ---------- main conv block: 18*nchunks matmuls per call ----------
            def mm_block(b, ot, r0, nchunks, ctag0=0):
                pcs = [psum.tile([P, 8, W], F32, name=f"pc{b}{ot}{r0}_{c}",
                                 tag=f"ch{(ctag0 + c) % 4}")
                       for c in range(nchunks)]
                first, last = (0, 0), (NI - 1, KK - 1)
                for it in range(NI):
                    for kk in range(KK):
                        dy, dx = kk // 3, kk % 3
                        lhsT = w_sb[it][:, kk, ot * P:(ot + 1) * P]
                        for c in range(nchunks):
                            rs = r0 + c * 8 + dy
                            nc.tensor.matmul(
                                pcs[c][:], lhsT, xs[(b, it)][:, rs:rs + 8, dx:dx + W],
                                start=((it, kk) == first), stop=((it, kk) == last),
                            )
                return pcs

            def out_block(b, ot, r0, pcs, engine="scalar"):
                n = len(pcs)
                oh = out_pool.tile([P, 8 * n, W], F32, name=f"oh{b}{ot}{r0}", tag="oh")
                sc = cols_sb[:, C_WINV + ot * B_LOC + b:C_WINV + ot * B_LOC + b + 1]
                bi = cols_sb[:, C_BIAS + ot:C_BIAS + ot + 1]
                for c in range(n):
                    if engine == "scalar":
                        nc.scalar.activation(
                            oh[:, c * 8:(c + 1) * 8, :], pcs[c][:], AF.Identity,
                            bias=bi, scale=sc,
                        )
                    else:
                        nc.vector.tensor_scalar(
                            oh[:, c * 8:(c + 1) * 8, :], pcs[c][:], sc, bi,
                            mybir.AluOpType.mult, mybir.AluOpType.add,
                        )
                nc.sync.dma_start(
                    out.ap()[b, ot * P:(ot + 1) * P, r0:r0 + 8 * n, :], oh[:])

            out_block(0, 0, 0, mm_block(0, 0, 0, 4))
            out_block(0, 0, 32, mm_block(0, 0, 32, 4))
            for half in range(2):
                out_block(0, 1, half * 32, mm_block(0, 1, half * 32, 4))

            # sample 1 input stage
            xs_borders(1)
            for it in range(NI):
                for part in range(2):
                    xs_cast(1, it, part)

            for half in range(2):
                out_block(1, 0, half * 32, mm_block(1, 0, half * 32, 4))
            out_block(1, 1, 0, mm_block(1, 1, 0, 4))
            # final blocks shrink progressively so the drain tail is short;
            # the last two drain ACT/DVE in parallel
            out_block(1, 1, 32, mm_block(1, 1, 32, 2, ctag0=0))
            out_block(1, 1, 48, mm_block(1, 1, 48, 1, ctag0=2))
            out_block(1, 1, 56, mm_block(1, 1, 56, 1, ctag0=3), engine="vector")
    return nc


_CACHED_NC = None


def _get_nc():
    global _CACHED_NC
    if _CACHED_NC is None:
        nc = bacc.Bacc("TRN2", target_bir_lowering=False, debug=False,
                       num_devices=N_CORES)
        build_conv2dmod(nc)
        nc.compile()
        _CACHED_NC = nc
    return _CACHED_NC


def kernel(x, y, weights, bias, style_w, style_b, _trace=False):
    x = np.asarray(x, dtype=np.float32)
    y = np.asarray(y, dtype=np.float32)
    weights = np.asarray(weights, dtype=np.float32)
    bias = np.asarray(bias, dtype=np.float32)
    style_w = np.asarray(style_w, dtype=np.float32)
    style_b = np.asarray(style_b, dtype=np.float32)

    # host-side layout prep + the tiny per-sample scalar pipeline (~4 MMAC)
    wt = np.ascontiguousarray(
        weights.transpose(1, 2, 3, 0).reshape(CIN, KK, COUT)
    ).astype(ml_dtypes.bfloat16)                                     # [i, kk, o]
    w2 = (weights * weights).sum(axis=(2, 3))                        # [o, i]
    style = y @ style_w.T + style_b                                  # [B, i]
    sigma = (style * style) @ w2.T                                   # [B, o]
    winv = 1.0 / np.sqrt(sigma + EPS)                                # [B, o]
    xbf = x.astype(ml_dtypes.bfloat16)

    nc = _get_nc()
    in_maps = []
    for c in range(N_CORES):
        sl = slice(c * B_LOC, (c + 1) * B_LOC)
        cc = np.empty((P, NCOLS), np.float32)
        # [B,2*P] -> [p, it/ot, b]
        cc[:, C_STYLE:C_STYLE + 4] = \
            style[sl].reshape(B_LOC, NI, P).transpose(2, 1, 0).reshape(P, 4)
        cc[:, C_WINV:C_WINV + 4] = \
            winv[sl].reshape(B_LOC, NO, P).transpose(2, 1, 0).reshape(P, 4)
        cc[:, C_BIAS:C_BIAS + 2] = bias.reshape(NO, P).T
        in_maps.append({
            "xbf": np.ascontiguousarray(xbf[sl]),
            "wt": wt,
            "cols": cc,
        })
    res = bass_utils.run_bass_kernel_spmd(
        nc, in_maps, core_ids=list(range(N_CORES)), trace=_trace
    )
    out = np.concatenate([r["out"] for r in res.results], axis=0)
    if _trace:
        kernel.last_results = res
    return out


# revision 8
# speedup vs baseline: 1.1286x; 1.0411x over previous
"""Modulated Conv2D (StyleGAN2-style) Trainium2 Bass kernel.

Problem shapes (hardcoded):
  x: [16, 256, 64, 64] f32    y: [16, 512] f32
  weights: [256, 256, 3, 3]   bias: [256]
  style_w: [256, 512]         style_b: [256]
  out: [16, 256, 64, 64] f32

Math identity used: instead of materializing per-sample modulated weights,
  out[b,o] = (1/wstd[b,o]) * conv(x[b] * style[b,:], w)[o] + bias[o]
  wstd[b,o] = sqrt(sum_i W2[o,i] * style[b,i]^2 + eps),  W2[o,i] = sum_kk w[o,i,kk]^2
so the conv weights are batch-independent (shared across samples/cores).

Sharding: data-parallel over batch, 2 samples per core across 8 cores.
Conv computed as 9 shifted matmuls (per 3x3 tap) accumulating in PSUM,
bf16 operands with f32 accumulation.

Host-side prep while sharding (all small or O(input-bytes) casts): weights
transposed to [cin, kk, cout] bf16, x cast to bf16 (halves HBM + doubles
DVE cast rate), and the per-sample scalars (style vector + demodulation
1/wstd, ~4 MMACs of the 77-GFLOP problem) folded into one packed [128,10]
f32 constant per core.  The device kernel is pure conv: DMA x over the two
HWDGE rings (inputs on the ACT ring, consts+outputs on the SYNC ring),
DVE scale-casts, PE streams 576 conv matmuls back-to-back (with a few
junk matmuls up front to lift the HAM clock gate early), ScalarE drains
scale+bias, stores overlap.
"""

import numpy as np
import ml_dtypes

import concourse.bass as bass
import concourse.tile as tile
from concourse import bacc, mybir
from concourse import bass_utils

EPS = 1e-8
P = 128
B_LOC = 2          # samples per core
CIN, COUT = 256, 256
NI, NO = CIN // P, COUT // P   # 2, 2
S = 512
H = W = 64
KK = 9             # 3x3 taps
HP, WP = H + 2, W + 2  # zero-padded image
N_CORES = 8
ROWS_A = 34        # first-chunk rows of the x load/cast split (covers half0 reads)
N_WARM_MM = 64     # junk matmuls to lift the HAM clock gate before the conv

F32 = mybir.dt.float32
BF16 = mybir.dt.bfloat16
AF = mybir.ActivationFunctionType

# packed per-core constant columns: [style(it,b) x4 | winv(ot,b) x4 | bias(ot) x2]
C_STYLE, C_WINV, C_BIAS, NCOLS = 0, 4, 8, 10


def build_conv2dmod(nc):
    xbf = nc.dram_tensor("xbf", [B_LOC, CIN, H, W], BF16, kind="ExternalInput")
    wt = nc.dram_tensor("wt", [CIN, KK, COUT], BF16, kind="ExternalInput")
    cols = nc.dram_tensor("cols", [P, NCOLS], F32, kind="ExternalInput")
    out = nc.dram_tensor("out", [B_LOC, COUT, H, W], F32, kind="ExternalOutput")

    with tile.TileContext(nc) as tc:
        with (
            tc.tile_pool(name="consts", bufs=1) as consts,
            tc.tile_pool(name="xin_pool", bufs=1) as xin_pool,
            tc.tile_pool(name="xs_pool", bufs=1) as xs_pool,
            tc.tile_pool(name="out_pool", bufs=3) as out_pool,
            tc.tile_pool(name="psum", bufs=2, space="PSUM") as psum,
        ):
            # ---------- DMA loads, split over both HWDGE rings ----------
            cols_sb = consts.tile([P, NCOLS], F32)
            nc.sync.dma_start(cols_sb[:], cols.ap())

            xin = {}
            for b in range(B_LOC):
                for it in range(NI):
                    xin[(b, it)] = xin_pool.tile([P, H, W], BF16,
                                                 name=f"xin{b}_{it}", tag=f"xin{b}_{it}")

            def load_xin(eng, b, it, part):
                r = slice(0, ROWS_A) if part == 0 else slice(ROWS_A, H)
                eng.dma_start(xin[(b, it)][:, r, :],
                              xbf.ap()[b, it * P:(it + 1) * P, r, :])

            w_sb = [consts.tile([P, KK, COUT], BF16, name=f"w_sb{i}", tag=f"w_sb{i}")
                    for i in range(NI)]
            # sync ring: first-needed x tile + second weights, then out stores.
            # act ring: first weights (split so the first taps land early) and
            # the remaining x tiles.  The two rings stream concurrently and
            # share HBM bandwidth, so order = priority.
            load_xin(nc.sync, 0, 0, 0)
            nc.scalar.dma_start(w_sb[0][:, 0:5, :], wt.ap()[0:P, 0:5, :])
            nc.scalar.dma_start(w_sb[0][:, 5:KK, :], wt.ap()[0:P, 5:KK, :])
            nc.sync.dma_start(w_sb[1][:], wt.ap()[P:2 * P])
            load_xin(nc.scalar, 0, 1, 0)
            load_xin(nc.sync, 0, 0, 1)
            load_xin(nc.scalar, 0, 1, 1)
            for it in range(NI):
                for part in range(2):
                    load_xin(nc.scalar, 1, it, part)

            # ---------- GPSIMD: pad borders;  ACT: warm the func table ----------
            lafs_warm = consts.tile([P, 1], F32)
            nc.scalar.activation(lafs_warm[:], cols_sb[:, 0:1], AF.Identity,
                                 bias=cols_sb[:, 1:2], scale=cols_sb[:, 2:3])

            xs = {}

            def xs_borders(b):
                for it in range(NI):
                    xp = xs_pool.tile([P, HP, WP], BF16, name=f"xs{b}_{it}",
                                      tag=f"xs{b}_{it}")
                    nc.gpsimd.memset(xp[:, 0, :], 0.0)
                    nc.gpsimd.memset(xp[:, HP - 1, :], 0.0)
                    nc.gpsimd.memset(xp[:, 1:HP - 1, 0], 0.0)
                    nc.gpsimd.memset(xp[:, 1:HP - 1, WP - 1], 0.0)
                    xs[(b, it)] = xp

            xs_borders(0)

            # ---------- PE warm-up: junk matmuls while x streams in ----------
            # fed from a memset tile (no DMA dependency) and sized to keep the
            # PE busy until the conv starts, so the HAM clock gate opens early
            # and stays open
            warm_src = consts.tile([P, 16], BF16)
            nc.gpsimd.memset(warm_src[:], 0.0)
            warm_ps = psum.tile([1, 16], F32, name="warm_ps", tag="ch0")
            for _ in range(N_WARM_MM):
                nc.tensor.matmul(warm_ps[:], warm_src[:, 0:1], warm_src[:],
                                 start=True, stop=True)

            # ---------- x scale+cast (DVE), ordered by need ----------
            def xs_cast(b, it, part):
                r = slice(0, ROWS_A) if part == 0 else slice(ROWS_A, H)
                return nc.vector.tensor_scalar_mul(
                    xs[(b, it)][:, r.start + 1:r.stop + 1, 1:W + 1],
                    xin[(b, it)][:, r, :],
                    cols_sb[:, C_STYLE + it * B_LOC + b:C_STYLE + it * B_LOC + b + 1],
                )

            xs_cast(0, 0, 0)
            xs_cast(0, 1, 0)
            xs_cast(0, 0, 1)
            xs_cast(0, 1, 1)

            # ---------- main conv block: 18*nchunks matmuls per call ----------
            def mm_block(b, ot, r0, nchunks, ctag0=0):
                pcs = [psum.tile([P, 8, W], F32, name=f"pc{b}{ot}{r0}_{c}",
                                 tag=f"ch{(ctag0 + c) % 4}")
                       for c in range(nchunks)]
                first, last = (0, 0), (NI - 1, KK - 1)
                for it in range(NI):
                    for kk in range(KK):
                        dy, dx = kk // 3, kk % 3
                        lhsT = w_sb[it][:, kk, ot * P:(ot + 1) * P]
                        for c in range(nchunks):
                            rs = r0 + c * 8 + dy
                            nc.tensor.matmul(
                                pcs[c][:], lhsT, xs[(b, it)][:, rs:rs + 8, dx:dx + W],
                                start=((it, kk) == first), stop=((it, kk) == last),
                            )
                return pcs

            def out_block(b, ot, r0, pcs, engine="scalar"):
                n = len(pcs)
                oh = out_pool.tile([P, 8 * n, W], F32, name=f"oh{b}{ot}{r0}", tag="oh")
                sc = cols_sb[:, C_WINV + ot * B_LOC + b:C_WINV + ot * B_LOC + b + 1]
                bi = cols_sb[:, C_BIAS + ot:C_BIAS + ot + 1]
                for c in range(n):
                    if engine == "scalar":
                        nc.scalar.activation(
                            oh[:, c * 8:(c + 1) * 8, :], pcs[c][:], AF.Identity,
                            bias=bi, scale=sc,
                        )
                    else:
                        nc.vector.tensor_scalar(
                            oh[:, c * 8:(c + 1) * 8, :], pcs[c][:], sc, bi,
                            mybir.AluOpType.mult, mybir.AluOpType.add,
                        )
                nc.sync.dma_start(
                    out.ap()[b, ot * P:(ot + 1) * P, r0:r0 + 8 * n, :], oh[:])

            out_block(0, 0, 0, mm_block(0, 0, 0, 4))
            out_block(0, 0, 32, mm_block(0, 0, 32, 4))
            for half in range(2):
                out_block(0, 1, half * 32, mm_block(0, 1, half * 32, 4))

            # sample 1 input stage
            xs_borders(1)
            for it in range(NI):
                for part in range(2):
                    xs_cast(1, it, part)

            for half in range(2):
                out_block(1, 0, half * 32, mm_block(1, 0, half * 32, 4))
            out_block(1, 1, 0, mm_block(1, 1, 0, 4))
            # final blocks shrink progressively so the drain tail is short;
            # the last two drain ACT/DVE in parallel
            out_block(1, 1, 32, mm_block(1, 1, 32, 2, ctag0=0))
            out_block(1, 1, 48, mm_block(1, 1, 48, 1, ctag0=2))
            out_block(1, 1, 56, mm_block(1, 1, 56, 1, ctag0=3), engine="vector")
    return nc


_CACHED_NC = None


def _get_nc():
    global _CACHED_NC
    if _CACHED_NC is None:
        nc = bacc.Bacc("TRN2", target_bir_lowering=False, debug=False,
                       num_devices=N_CORES)
        build_conv2dmod(nc)
        nc.compile()
        _CACHED_NC = nc
    return _CACHED_NC


def kernel(x, y, weights, bias, style_w, style_b, _trace=False):
    x = np.asarray(x, dtype=np.float32)
    y = np.asarray(y, dtype=np.float32)
    weights = np.asarray(weights, dtype=np.float32)
    bias = np.asarray(bias, dtype=np.float32)
    style_w = np.asarray(style_w, dtype=np.float32)
    style_b = np.asarray(style_b, dtype=np.float32)

    # host-side layout prep + the tiny per-sample scalar pipeline (~4 MMAC)
    wt = np.ascontiguousarray(
        weights.transpose(1, 2, 3, 0).reshape(CIN, KK, COUT)
    ).astype(ml_dtypes.bfloat16)                                     # [i, kk, o]
    w2 = (weights * weights).sum(axis=(2, 3))                        # [o, i]
    style = y @ style_w.T + style_b                                  # [B, i]
    sigma = (style * style) @ w2.T                                   # [B, o]
    winv = 1.0 / np.sqrt(sigma + EPS)                                # [B, o]
    xbf = x.astype(ml_dtypes.bfloat16)

    nc = _get_nc()
    in_maps = []
    for c in range(N_CORES):
        sl = slice(c * B_LOC, (c + 1) * B_LOC)
        cc = np.empty((P, NCOLS), np.float32)
        # [B,2*P] -> [p, it/ot, b]
        cc[:, C_STYLE:C_STYLE + 4] = \
            style[sl].reshape(B_LOC, NI, P).transpose(2, 1, 0).reshape(P, 4)
        cc[:, C_WINV:C_WINV + 4] = \
            winv[sl].reshape(B_LOC, NO, P).transpose(2, 1, 0).reshape(P, 4)
        cc[:, C_BIAS:C_BIAS + 2] = bias.reshape(NO, P).T
        in_maps.append({
            "xbf": np.ascontiguousarray(xbf[sl]),
            "wt": wt,
            "cols": cc,
        })
    res = bass_utils.run_bass_kernel_spmd(
        nc, in_maps, core_ids=list(range(N_CORES)), trace=_trace
    )
    out = np.concatenate([r["out"] for r in res.results], axis=0)
    if _trace:
        kernel.last_results = res
    return out


# revision 21
# speedup vs baseline: 1.1442x; 1.0138x over previous
"""Modulated Conv2D (StyleGAN2-style) Trainium2 Bass kernel.

Problem shapes (hardcoded):
  x: [16, 256, 64, 64] f32    y: [16, 512] f32
  weights: [256, 256, 3, 3]   bias: [256]
  style_w: [256, 512]         style_b: [256]
  out: [16, 256, 64, 64] f32

Math identity used: instead of materializing per-sample modulated weights on
device, the modulation and demodulation are folded into the weights on the
host while sharding:
  out[b,o] = conv(x[b], w * style[b,i] * winv[b,o])[o] + bias[o]
  winv[b,o] = 1/sqrt(sum_i W2[o,i] * style[b,i]^2 + eps)
(style/winv are ~4 MMACs of the 77-GFLOP problem).  The device kernel is
pure conv: x (bf16, host-cast) DMAs straight into zero-row-padded SBUF
tiles, the PE streams 9-tap shifted matmuls back-to-back accumulating in
PSUM, ScalarE adds bias on the drains, stores overlap.

Column padding is avoided entirely: the dx=0/dx=2 taps write 63-wide PSUM
column sub-ranges (the out-of-image contribution is zero), with a full-width
dx=1 tap ordered first in each accumulation group so every PSUM element is
initialized.

Sharding: data-parallel over batch, 2 samples per core across 8 cores.
Inputs stream over both HWDGE rings (order = priority); a dozen junk
matmuls up front lift the HAM clock gate before the conv arrives.
"""

import numpy as np
import ml_dtypes

import concourse.bass as bass
import concourse.tile as tile
from concourse import bacc, mybir
from concourse import bass_utils

EPS = 1e-8
P = 128
B_LOC = 2          # samples per core
CIN, COUT = 256, 256
NI, NO = CIN // P, COUT // P   # 2, 2
S = 512
H = W = 64
KK = 9             # 3x3 taps
HP = H + 2         # zero-padded rows (columns are not padded)
N_CORES = 8
ROWS_A = 34        # first-chunk rows of the b=1 x load split
N_WARM_MM = 12     # junk matmuls to lift the HAM clock gate before the conv
WARM_COLS = 512    # columns per junk matmul (paces the warm-up)

F32 = mybir.dt.float32
BF16 = mybir.dt.bfloat16
AF = mybir.ActivationFunctionType

# tap order within an accumulation group: a full-width dx=1 tap first so the
# start=True matmul initializes every PSUM element of the chunk
KK_ORDER = [1, 0, 2, 3, 4, 5, 6, 7, 8]
# per-dx (rhs column range, psum column range)
DX_COLS = {0: (slice(0, W - 1), slice(1, W)),
           1: (slice(0, W), slice(0, W)),
           2: (slice(1, W), slice(0, W - 1))}


def build_conv2dmod(nc):
    xbf = nc.dram_tensor("xbf", [B_LOC, CIN, H, W], BF16, kind="ExternalInput")
    wf = nc.dram_tensor("wf", [B_LOC, CIN, KK, COUT], BF16, kind="ExternalInput")
    bias_col = nc.dram_tensor("bias_col", [P, NO], F32, kind="ExternalInput")
    out = nc.dram_tensor("out", [B_LOC, COUT, H, W], F32, kind="ExternalOutput")

    with tile.TileContext(nc) as tc:
        with (
            tc.tile_pool(name="consts", bufs=1) as consts,
            tc.tile_pool(name="xs_pool", bufs=1) as xs_pool,
            tc.tile_pool(name="out_pool", bufs=3) as out_pool,
            tc.tile_pool(name="psum", bufs=2, space="PSUM") as psum,
        ):
            bias_sb = consts.tile([P, NO], F32)
            nc.sync.dma_start(bias_sb[:], bias_col.ap())

            # x tiles: row-padded only, so both the DMA source and the SBUF
            # destination are fully contiguous per partition (line-rate DMA)
            xs = {}
            for b in range(B_LOC):
                for it in range(NI):
                    xs[(b, it)] = xs_pool.tile([P, HP, W], BF16,
                                               name=f"xs{b}_{it}", tag=f"xs{b}_{it}")

            def load_x(eng, b, it, r):
                eng.dma_start(xs[(b, it)][:, r.start + 1:r.stop + 1, :],
                              xbf.ap()[b, it * P:(it + 1) * P, r, :])

            w_sb = {}
            for b in range(B_LOC):
                for it in range(NI):
                    w_sb[(b, it)] = consts.tile([P, KK, COUT], BF16,
                                                name=f"w{b}_{it}", tag=f"w{b}_{it}")

            def load_w(eng, b, it, k0, k1):
                eng.dma_start(w_sb[(b, it)][:, k0:k1, :],
                              wf.ap()[b, it * P:(it + 1) * P, k0:k1, :])

            # load order = priority; sample-0 x in three row slices so the
            # first conv sub-block can start as early as possible
            RB = (slice(0, 17), slice(17, ROWS_A), slice(ROWS_A, H))
            load_x(nc.sync, 0, 0, RB[0])
            load_x(nc.scalar, 0, 1, RB[0])
            load_w(nc.scalar, 0, 0, 0, 5)
            load_x(nc.sync, 0, 0, RB[1])
            load_w(nc.scalar, 0, 0, 5, KK)
            load_w(nc.sync, 0, 1, 0, 5)
            load_x(nc.scalar, 0, 1, RB[1])
            load_w(nc.sync, 0, 1, 5, KK)
            load_x(nc.sync, 0, 0, RB[2])
            load_x(nc.scalar, 0, 1, RB[2])
            load_x(nc.scalar, 1, 0, slice(0, ROWS_A))
            load_x(nc.scalar, 1, 0, slice(ROWS_A, H))
            load_x(nc.scalar, 1, 1, slice(0, ROWS_A))
            load_x(nc.scalar, 1, 1, slice(ROWS_A, H))
            load_w(nc.sync, 1, 0, 0, KK)
            load_w(nc.scalar, 1, 1, 0, KK)

            # ---------- ACT func-table warm + pad-row memsets ----------
            lafs_warm = consts.tile([P, 1], F32)
            nc.scalar.activation(lafs_warm[:], bias_sb[:, 0:1], AF.Identity,
                                 bias=bias_sb[:, 1:2])

            # ---------- PE warm-up: junk matmuls while x streams in ----------
            warm_src = consts.tile([P, WARM_COLS], BF16)
            nc.gpsimd.memset(warm_src[:], 0.0)
            warm_ps = psum.tile([1, WARM_COLS], F32, name="warm_ps", tag="ch0")
            for _ in range(N_WARM_MM):
                nc.tensor.matmul(warm_ps[:], warm_src[:, 0:1], warm_src[:],
                                 start=True, stop=True)

            for b in range(B_LOC):
                for it in range(NI):
                    nc.gpsimd.memset(xs[(b, it)][:, 0, :], 0.0)
                    nc.gpsimd.memset(xs[(b, it)][:, HP - 1, :], 0.0)

            # ---------- main conv block: 18*nchunks matmuls per call ----------
            def mm_block(b, ot, r0, nchunks, ctag0=0):
                pcs = [psum.tile([P, 8, W], F32, name=f"pc{b}{ot}{r0}_{c}",
                                 tag=f"ch{(ctag0 + c) % 4}")
                       for c in range(nchunks)]
                first, last = (0, KK_ORDER[0]), (NI - 1, KK_ORDER[-1])
                for it in range(NI):
                    for kk in KK_ORDER:
                        dy, dx = kk // 3, kk % 3
                        rc, oc = DX_COLS[dx]
                        lhsT = w_sb[(b, it)][:, kk, ot * P:(ot + 1) * P]
                        for c in range(nchunks):
                            rs = r0 + c * 8 + dy
                            nc.tensor.matmul(
                                pcs[c][:, :, oc], lhsT,
                                xs[(b, it)][:, rs:rs + 8, rc],
                                start=((it, kk) == first), stop=((it, kk) == last),
                            )
                return pcs

            def out_block(b, ot, r0, pcs, engine="scalar", ring=None):
                n = len(pcs)
                oh = out_pool.tile([P, 8 * n, W], F32, name=f"oh{b}{ot}{r0}", tag="oh")
                bi = bias_sb[:, ot:ot + 1]
                for c in range(n):
                    if engine == "scalar":
                        nc.scalar.activation(
                            oh[:, c * 8:(c + 1) * 8, :], pcs[c][:], AF.Identity,
                            bias=bi,
                        )
                    else:
                        nc.vector.tensor_scalar_add(
                            oh[:, c * 8:(c + 1) * 8, :], pcs[c][:], bi)
                (ring or nc.sync).dma_start(
                    out.ap()[b, ot * P:(ot + 1) * P, r0:r0 + 8 * n, :], oh[:])

            # first two sub-blocks are 16-row so the conv starts as soon as
            # the first 17-row x slice lands
            out_block(0, 0, 0, mm_block(0, 0, 0, 2, ctag0=0))
            out_block(0, 0, 16, mm_block(0, 0, 16, 2, ctag0=2))
            out_block(0, 0, 32, mm_block(0, 0, 32, 4))
            for half in range(2):
                out_block(0, 1, half * 32, mm_block(0, 1, half * 32, 4))
            for half in range(2):
                out_block(1, 0, half * 32, mm_block(1, 0, half * 32, 4))
            out_block(1, 1, 0, mm_block(1, 1, 0, 4))
            # final blocks shrink progressively so the drain tail is short;
            # the last two drain ACT/DVE and store on both rings in parallel
            out_block(1, 1, 32, mm_block(1, 1, 32, 2, ctag0=0))
            out_block(1, 1, 48, mm_block(1, 1, 48, 1, ctag0=2), ring=nc.scalar)
            out_block(1, 1, 56, mm_block(1, 1, 56, 1, ctag0=3), engine="vector")
    return nc


_CACHED_NC = None
_PREP_JIT = None


def _get_nc():
    global _CACHED_NC
    if _CACHED_NC is None:
        nc = bacc.Bacc("TRN2", target_bir_lowering=False, debug=False,
                       num_devices=N_CORES)
        build_conv2dmod(nc)
        nc.compile()
        _CACHED_NC = nc
    return _CACHED_NC


def _get_prep():
    """jit'd host-side prep on jax-cpu (multithreaded): x -> bf16 and the
    folded per-sample conv weights wf[b,i,kk,o] = w*style*winv in bf16."""
    global _PREP_JIT
    if _PREP_JIT is None:
        import jax
        import jax.numpy as jnp

        cpu = jax.devices("cpu")[0]

        def _prep(x, y, weights, bias, style_w, style_b):
            style = y @ style_w.T + style_b                       # [B, i]
            w2 = jnp.sum(weights * weights, axis=(2, 3))          # [o, i]
            sigma = (style * style) @ w2.T                        # [B, o]
            winv = 1.0 / jnp.sqrt(sigma + EPS)                    # [B, o]
            wfull = (weights[None] * style[:, None, :, None, None]
                     * winv[:, :, None, None, None])              # [B,o,i,3,3]
            wf = wfull.transpose(0, 2, 3, 4, 1).reshape(
                len(style), CIN, KK, COUT).astype(jnp.bfloat16)
            return x.astype(jnp.bfloat16), wf

        _PREP_JIT = (jax.jit(_prep, device=cpu), cpu)
    return _PREP_JIT


def kernel(x, y, weights, bias, style_w, style_b, _trace=False):
    x = np.asarray(x, dtype=np.float32)
    y = np.asarray(y, dtype=np.float32)
    weights = np.asarray(weights, dtype=np.float32)
    bias = np.asarray(bias, dtype=np.float32)
    style_w = np.asarray(style_w, dtype=np.float32)
    style_b = np.asarray(style_b, dtype=np.float32)

    prep, cpu = _get_prep()
    import jax
    with jax.default_device(cpu):
        xbf, wfj = prep(x, y, weights, bias, style_w, style_b)
        xbf = np.asarray(xbf)
        wfn = np.asarray(wfj)
    bias_c = np.ascontiguousarray(bias.reshape(NO, P).T)           # [p, oo]

    nc = _get_nc()
    in_maps = []
    for c in range(N_CORES):
        sl = slice(c * B_LOC, (c + 1) * B_LOC)
        in_maps.append({
            "xbf": np.ascontiguousarray(xbf[sl]),
            "wf": np.ascontiguousarray(wfn[sl]),
            "bias_col": bias_c,
        })
    res = bass_utils.run_bass_kernel_spmd(
        nc, in_maps, core_ids=list(range(N_CORES)), trace=_trace
    )
    out = np.concatenate([r["out"] for r in res.results], axis=0)
    if _trace:
        kernel.last_results = res
    return out
